# revision 9
# baseline (speedup 1.0000x reference)
"""Margin-softmax head (ArcFace-style) distributed over 8 TRN2 NeuronCores.

out = S * cosine, except out[i, label[i]] = S * (-A*acos(cosine[i, label[i]]) + B)
for rows with a valid label. Class columns are sharded 8 ways (partial-FC).

The bulk path is pure memory-bound (one multiply per element), and the cost
is DMA bytes: rel tolerance is 2e-2, so the bulk tensor is staged in DRAM as
bf16 (round-to-nearest from f32; <= 2^-9 relative error) and the output is
written as bf16 too (upcast to f32 on host after the gather). That halves
DMA traffic vs f32 -> ~2x on the 360 GB/s per-core DMA roofline.

acos near x=1 is ill-conditioned (d/dx = -1/sqrt(1-x^2)), so the <=512
target elements are gathered from a full-precision f32 copy of the shard
(staged alongside; only 512 elements of it are ever read on device). The
margin pipeline runs in f32 and converts to bf16 only at the final affine
step, then indirect-scatters into the bf16 output (OOB sentinel rows are
silently skipped via bounds_check).

acos(x) = 2*atan(sqrt((1-x)/(1+x))), well conditioned on (-1, 1].
"""

from contextlib import ExitStack

import numpy as np

import concourse.bacc as bacc
import concourse.bass as bass
import concourse.mybir as mybir
from concourse.bass_utils import run_bass_kernel_spmd
from concourse.tile import TileContext

try:
    import ml_dtypes

    BF16_NP = np.dtype(ml_dtypes.bfloat16)
except ImportError:  # pragma: no cover
    BF16_NP = np.dtype("bfloat16")

A = 0.88
B = 0.88
S = 64.0

BATCH = 512
NUM_CLASSES = 100000
NCORES = 8
SHARD = NUM_CLASSES // NCORES  # 12500
ROW_CHUNKS = BATCH // 128  # 4
NELEM = BATCH * SHARD  # flat elements per shard
OOB_SENTINEL = NELEM + 1  # > bounds_check -> transfer silently skipped

F32 = mybir.dt.float32
BF16 = mybir.dt.bfloat16
I32 = mybir.dt.int32

_NC = None
LAST_RESULT = None  # BassKernelResults of the most recent run (for test harness)


def _build_nc(col_tile=6250, bufs=6, engine="vector", margin=True, repeat=1):
    nc = bacc.Bacc("TRN2", target_bir_lowering=False, debug=False)

    cos16 = nc.declare_dram_parameter("cos16", [BATCH, SHARD], BF16, isOutput=False)
    cosf = nc.declare_dram_parameter("cosf", [BATCH, SHARD], F32, isOutput=False)
    idx = nc.declare_dram_parameter("idx", [128, ROW_CHUNKS], I32, isOutput=False)
    out = nc.declare_dram_parameter("out", [BATCH, SHARD], BF16, isOutput=True)

    n_col_tiles = SHARD // col_tile
    assert SHARD % col_tile == 0

    with TileContext(nc) as tc:
        with ExitStack() as stack:
          pool = stack.enter_context(tc.tile_pool(name="bulk", bufs=bufs))
          sp = (stack.enter_context(tc.tile_pool(name="small", bufs=1))
                if margin else None)
          for _rep in range(repeat):
            if margin:
                # ---- margin fix-up path (tiny, runs concurrently w/ bulk) ----
                idx_sb = sp.tile([128, ROW_CHUNKS], I32)
                nc.sync.dma_start(out=idx_sb[:], in_=idx[:])

                gx = sp.tile([128, ROW_CHUNKS], F32)
                nc.vector.memset(gx[:], 0.0)
                # gather cosine[i, label_i] from the f32 copy (flat element
                # index, coef=1 on axis 1). NOTE: HW pairs ONE index per
                # partition with the whole free-dim run of the data AP, so
                # these must stay [128, 1] per transfer (indirect DMA requires
                # the DRAM-side AP at offset 0, so indices are global-flat).
                for r in range(ROW_CHUNKS):
                    nc.gpsimd.indirect_dma_start(
                        out=gx[:, r : r + 1],
                        out_offset=None,
                        in_=cosf[:],
                        in_offset=bass.IndirectOffsetOnAxis(
                            ap=idx_sb[:, r : r + 1], axis=1
                        ),
                        bounds_check=NELEM - 1,
                        oob_is_err=False,
                    )

                num = sp.tile([128, ROW_CHUNKS], F32)
                den = sp.tile([128, ROW_CHUNKS], F32)
                val = sp.tile([128, ROW_CHUNKS], F32)
                val16 = sp.tile([128, ROW_CHUNKS], BF16)
                # num = 1 - x ; den = 1 + x ; val = num/den
                nc.vector.tensor_scalar(num[:], gx[:], -1.0, 1.0,
                                        mybir.AluOpType.mult, mybir.AluOpType.add)
                nc.vector.tensor_scalar_add(den[:], gx[:], 1.0)
                nc.vector.reciprocal(den[:], den[:])
                nc.vector.tensor_tensor(out=val[:], in0=num[:], in1=den[:],
                                        op=mybir.AluOpType.mult)
                # val = atan(sqrt(val)) ; then affine (+ f32->bf16 convert):
                # S*(-A*2*atan + B)
                nc.scalar.activation(val[:], val[:],
                                     mybir.ActivationFunctionType.Sqrt)
                nc.scalar.activation(val[:], val[:],
                                     mybir.ActivationFunctionType.Arctan)
                nc.scalar.activation(val16[:], val[:],
                                     mybir.ActivationFunctionType.Copy,
                                     bias=S * B, scale=-2.0 * S * A)

            # ---- bulk scale pass (bf16 in, bf16 out) ----
            cos_t = cos16[:].rearrange("(r p) m -> r p m", p=128)
            out_t = out[:].rearrange("(r p) m -> r p m", p=128)
            for r in range(ROW_CHUNKS):
                for j in range(n_col_tiles):
                    t = pool.tile([128, col_tile], BF16)
                    cs = slice(j * col_tile, (j + 1) * col_tile)
                    nc.sync.dma_start(out=t[:], in_=cos_t[r, :, cs])
                    if engine == "vector":
                        nc.vector.tensor_scalar_mul(t[:], t[:], S)
                    elif engine == "scalar":
                        nc.scalar.mul(t[:], t[:], S)
                    else:  # alternate
                        if (r * n_col_tiles + j) % 2 == 0:
                            nc.vector.tensor_scalar_mul(t[:], t[:], S)
                        else:
                            nc.scalar.mul(t[:], t[:], S)
                    nc.sync.dma_start(out=out_t[r, :, cs], in_=t[:])

            if margin:
                # ---- scatter fix-up (ordered after all bulk writes, WAW) ----
                # [128, 1] per transfer: same one-index-per-partition HW rule.
                for r in range(ROW_CHUNKS):
                    nc.gpsimd.indirect_dma_start(
                        out=out[:],
                        out_offset=bass.IndirectOffsetOnAxis(
                            ap=idx_sb[:, r : r + 1], axis=1
                        ),
                        in_=val16[:, r : r + 1],
                        in_offset=None,
                        bounds_check=NELEM - 1,
                        oob_is_err=False,
                    )

    nc.compile()
    return nc


def _in_maps(cosine: np.ndarray, label: np.ndarray):
    cosine = np.asarray(cosine, dtype=np.float32)
    cosine16 = cosine.astype(BF16_NP)
    label = np.asarray(label)
    rows = np.arange(BATCH, dtype=np.int64)
    in_maps = []
    for c in range(NCORES):
        lo = c * SHARD
        shard_f = np.ascontiguousarray(cosine[:, lo : lo + SHARD])
        shard16 = np.ascontiguousarray(cosine16[:, lo : lo + SHARD])
        loc = label.astype(np.int64) - lo
        valid = (label != -1) & (loc >= 0) & (loc < SHARD)
        flat = np.where(valid, rows * SHARD + loc, OOB_SENTINEL).astype(np.int32)
        # device layout: idx[p, r] = flat[r*128 + p]
        idx_dev = np.ascontiguousarray(flat.reshape(ROW_CHUNKS, 128).T)
        in_maps.append({"cos16": shard16, "cosf": shard_f, "idx": idx_dev})
    return in_maps


def kernel(cosine: np.ndarray, label: np.ndarray) -> np.ndarray:
    global _NC, LAST_RESULT
    if _NC is None:
        _NC = _build_nc()
    res = run_bass_kernel_spmd(_NC, _in_maps(cosine, label),
                               core_ids=list(range(NCORES)))
    LAST_RESULT = res
    out16 = np.concatenate([res.results[c]["out"] for c in range(NCORES)], axis=1)
    return out16.astype(np.float32)


# revision 19
# speedup vs baseline: 1.0039x; 1.0039x over previous
"""Margin-softmax head (ArcFace-style) distributed over 8 TRN2 NeuronCores.

out = S * cosine, except out[i, label[i]] = S * (-A*acos(cosine[i, label[i]]) + B)
for rows with a valid label. Class columns are sharded 8 ways (partial-FC).

The bulk path is pure memory-bound (one multiply per element), and the cost
is DMA bytes: rel tolerance is 2e-2, so the bulk tensor is staged in DRAM as
bf16 (round-to-nearest from f32; <= 2^-9 relative error) and the output is
written as bf16 too (upcast to f32 on host after the gather). That halves
DMA traffic vs f32 -> ~2x on the 360 GB/s per-core DMA roofline.

acos near x=1 is ill-conditioned (d/dx = -1/sqrt(1-x^2)), so the <=512
target elements are gathered from a full-precision f32 copy of the shard
(staged alongside; only 512 elements of it are ever read on device). The
margin pipeline runs in f32 and converts to bf16 only at the final affine
step, then indirect-scatters into the bf16 output (OOB sentinel rows are
silently skipped via bounds_check).

acos(x) = 2*atan(sqrt((1-x)/(1+x))), well conditioned on (-1, 1].
"""

from contextlib import ExitStack

import numpy as np

import concourse.bacc as bacc
import concourse.bass as bass
import concourse.mybir as mybir
from concourse.bass_utils import run_bass_kernel_spmd
from concourse.tile import TileContext

try:
    import ml_dtypes

    BF16_NP = np.dtype(ml_dtypes.bfloat16)
except ImportError:  # pragma: no cover
    BF16_NP = np.dtype("bfloat16")

A = 0.88
B = 0.88
S = 64.0

BATCH = 512
NUM_CLASSES = 100000
NCORES = 8
SHARD = NUM_CLASSES // NCORES  # 12500
ROW_CHUNKS = BATCH // 128  # 4
NELEM = BATCH * SHARD  # flat elements per shard
OOB_SENTINEL = NELEM + 1  # > bounds_check -> transfer silently skipped

F32 = mybir.dt.float32
BF16 = mybir.dt.bfloat16
I32 = mybir.dt.int32

_NC = None
LAST_RESULT = None  # BassKernelResults of the most recent run (for test harness)


def _build_nc(col_tile=6250, bufs=6, engine="vector", margin=True, repeat=1):
    nc = bacc.Bacc("TRN2", target_bir_lowering=False, debug=False)

    cos16 = nc.declare_dram_parameter("cos16", [BATCH, SHARD], BF16, isOutput=False)
    cosf = nc.declare_dram_parameter("cosf", [BATCH, SHARD], F32, isOutput=False)
    idx = nc.declare_dram_parameter("idx", [128, ROW_CHUNKS], I32, isOutput=False)
    out = nc.declare_dram_parameter("out", [BATCH, SHARD], BF16, isOutput=True)

    n_col_tiles = SHARD // col_tile
    assert SHARD % col_tile == 0

    with TileContext(nc) as tc:
        with ExitStack() as stack:
          pool = stack.enter_context(tc.tile_pool(name="bulk", bufs=bufs))
          sp = (stack.enter_context(tc.tile_pool(name="small", bufs=1))
                if margin else None)
          for _rep in range(repeat):
            if margin:
                # ---- margin fix-up path (tiny, runs concurrently w/ bulk) ----
                idx_sb = sp.tile([128, ROW_CHUNKS], I32)
                nc.sync.dma_start(out=idx_sb[:], in_=idx[:])

                gx = sp.tile([128, ROW_CHUNKS], F32)
                nc.vector.memset(gx[:], 0.0)
                # gather cosine[i, label_i] from the f32 copy (flat element
                # index, coef=1 on axis 1). NOTE: HW pairs ONE index per
                # partition with the whole free-dim run of the data AP, so
                # these must stay [128, 1] per transfer (indirect DMA requires
                # the DRAM-side AP at offset 0, so indices are global-flat).
                for r in range(ROW_CHUNKS):
                    nc.gpsimd.indirect_dma_start(
                        out=gx[:, r : r + 1],
                        out_offset=None,
                        in_=cosf[:],
                        in_offset=bass.IndirectOffsetOnAxis(
                            ap=idx_sb[:, r : r + 1], axis=1
                        ),
                        bounds_check=NELEM - 1,
                        oob_is_err=False,
                    )

                num = sp.tile([128, ROW_CHUNKS], F32)
                den = sp.tile([128, ROW_CHUNKS], F32)
                val = sp.tile([128, ROW_CHUNKS], F32)
                val16 = sp.tile([128, ROW_CHUNKS], BF16)
                # num = 1 - x ; den = 1 + x ; val = num/den
                nc.vector.tensor_scalar(num[:], gx[:], -1.0, 1.0,
                                        mybir.AluOpType.mult, mybir.AluOpType.add)
                nc.vector.tensor_scalar_add(den[:], gx[:], 1.0)
                nc.vector.reciprocal(den[:], den[:])
                nc.vector.tensor_tensor(out=val[:], in0=num[:], in1=den[:],
                                        op=mybir.AluOpType.mult)
                # val = atan(sqrt(val)) ; then affine (+ f32->bf16 convert):
                # S*(-A*2*atan + B)
                nc.scalar.activation(val[:], val[:],
                                     mybir.ActivationFunctionType.Sqrt)
                nc.scalar.activation(val[:], val[:],
                                     mybir.ActivationFunctionType.Arctan)
                nc.scalar.activation(val16[:], val[:],
                                     mybir.ActivationFunctionType.Copy,
                                     bias=S * B, scale=-2.0 * S * A)

            # ---- bulk scale pass (bf16 in, bf16 out) ----
            cos_t = cos16[:].rearrange("(r p) m -> r p m", p=128)
            out_t = out[:].rearrange("(r p) m -> r p m", p=128)
            for r in range(ROW_CHUNKS):
                for j in range(n_col_tiles):
                    t = pool.tile([128, col_tile], BF16)
                    cs = slice(j * col_tile, (j + 1) * col_tile)
                    nc.sync.dma_start(out=t[:], in_=cos_t[r, :, cs])
                    if engine == "vector":
                        nc.vector.tensor_scalar_mul(t[:], t[:], S)
                    elif engine == "scalar":
                        nc.scalar.mul(t[:], t[:], S)
                    else:  # alternate
                        if (r * n_col_tiles + j) % 2 == 0:
                            nc.vector.tensor_scalar_mul(t[:], t[:], S)
                        else:
                            nc.scalar.mul(t[:], t[:], S)
                    nc.sync.dma_start(out=out_t[r, :, cs], in_=t[:])

            if margin:
                # ---- scatter fix-up (ordered after all bulk writes, WAW) ----
                # [128, 1] per transfer: same one-index-per-partition HW rule.
                for r in range(ROW_CHUNKS):
                    nc.gpsimd.indirect_dma_start(
                        out=out[:],
                        out_offset=bass.IndirectOffsetOnAxis(
                            ap=idx_sb[:, r : r + 1], axis=1
                        ),
                        in_=val16[:, r : r + 1],
                        in_offset=None,
                        bounds_check=NELEM - 1,
                        oob_is_err=False,
                    )

    nc.compile()
    return nc


def _build_raw(col_tile=6250, margin=True, repeat=1, dbg=False):
    """Hand-scheduled variant of _build_nc: no TileContext, explicit
    semaphores. Same dataflow and the same DMA schedule the Tile version
    converges to (L0 L1 S0 L2 S1 ... — stores lag loads by 2 in DMA-queue
    order), but without the Tile prologue barrier / epilogue, which are the
    only non-roofline items left on the critical path.

    Protocol (per 128-row x col_tile tile g, numbered across repeats):
      SP:   [WAR: st >= 16*(g-NB+1)] load -> buf[g%NB], +16 ld
            [cp >= g-1] store buf[(g-2)%NB], +16 st   (lag-2 interleave)
      DVE:  [ld >= 16*(g+1)] buf *= S, +1 cp
    Margin: idx DMA (+16 ix) -> Pool gathers after memset (+16 gt each) ->
    DVE rational + Act sqrt/atan/affine (+1 mv/mact) -> Pool scatters after
    all stores (+16 sc each). Final SP waits pin down DMA completion.
    """
    nc = bacc.Bacc("TRN2", target_bir_lowering=False, debug=False)

    cos16 = nc.declare_dram_parameter("cos16", [BATCH, SHARD], BF16, isOutput=False)
    cosf = nc.declare_dram_parameter("cosf", [BATCH, SHARD], F32, isOutput=False)
    idx = nc.declare_dram_parameter("idx", [128, ROW_CHUNKS], I32, isOutput=False)
    out = nc.declare_dram_parameter("out", [BATCH, SHARD], BF16, isOutput=True)

    if dbg:
        dbg_gx = nc.declare_dram_parameter("dbg_gx", [128, ROW_CHUNKS], F32,
                                           isOutput=True)
        dbg_ix = nc.declare_dram_parameter("dbg_ix", [128, ROW_CHUNKS], I32,
                                           isOutput=True)
        dbg_v16 = nc.declare_dram_parameter("dbg_v16", [128, ROW_CHUNKS], BF16,
                                            isOutput=True)

    n_col_tiles = SHARD // col_tile
    assert SHARD % col_tile == 0
    n_tiles = ROW_CHUNKS * n_col_tiles
    NB = min(6, n_tiles)
    LAG = 2
    assert n_tiles >= 4 and NB > LAG

    cos_t = cos16[:].rearrange("(r p) m -> r p m", p=128)
    out_t = out[:].rearrange("(r p) m -> r p m", p=128)

    def tile_ap(ap3, g):
        r, j = divmod(g % n_tiles, n_col_tiles)
        return ap3[r, :, j * col_tile : (j + 1) * col_tile]

    with ExitStack() as es:
        bufs = [es.enter_context(nc.sbuf_tensor(f"buf{b}", [128, col_tile], BF16))
                for b in range(NB)]
        ld = es.enter_context(nc.semaphore("ld"))
        st = es.enter_context(nc.semaphore("st"))
        cp = es.enter_context(nc.semaphore("cp"))
        if margin:
            idx_sb = es.enter_context(nc.sbuf_tensor([128, ROW_CHUNKS], I32))
            gx = es.enter_context(nc.sbuf_tensor([128, ROW_CHUNKS], F32))
            num = es.enter_context(nc.sbuf_tensor([128, ROW_CHUNKS], F32))
            den = es.enter_context(nc.sbuf_tensor([128, ROW_CHUNKS], F32))
            val = es.enter_context(nc.sbuf_tensor([128, ROW_CHUNKS], F32))
            val16 = es.enter_context(nc.sbuf_tensor([128, ROW_CHUNKS], BF16))
            ix = es.enter_context(nc.semaphore("ix"))
            mz = es.enter_context(nc.semaphore("mz"))
            gt = es.enter_context(nc.semaphore("gt"))
            dv = es.enter_context(nc.semaphore("dv"))
            ac = es.enter_context(nc.semaphore("ac"))
            sc = es.enter_context(nc.semaphore("sc"))

        with nc.Block() as block:

            @block.sync
            def _(sync):
                for rep in range(repeat):
                    for i in range(n_tiles):
                        g = rep * n_tiles + i
                        if g >= NB:
                            # WAR: buf[g%NB] free once store g-NB completed
                            sync.wait_ge(st, 16 * (g - NB + 1))
                        sync.dma_start(out=bufs[g % NB][:],
                                       in_=tile_ap(cos_t, g)).then_inc(ld, 16)
                        if margin and i == 2:
                            if rep:
                                # idx_sb/val16 still read by rep-1 scatters
                                sync.wait_ge(sc, 64 * rep)
                            sync.dma_start(out=idx_sb[:],
                                           in_=idx[:]).then_inc(ix, 16)
                        if i >= LAG:
                            g_s = g - LAG
                            sync.wait_ge(cp, g_s + 1)
                            sync.dma_start(out=tile_ap(out_t, g_s),
                                           in_=bufs[g_s % NB][:]).then_inc(st, 16)
                    for g_s in range(rep * n_tiles + n_tiles - LAG,
                                     (rep + 1) * n_tiles):
                        sync.wait_ge(cp, g_s + 1)
                        sync.dma_start(out=tile_ap(out_t, g_s),
                                       in_=bufs[g_s % NB][:]).then_inc(st, 16)
                # pin DMA completion of everything the program must finish
                sync.wait_ge(st, 16 * n_tiles * repeat)
                if margin:
                    sync.wait_ge(sc, 64 * repeat)
                    if dbg:
                        sync.dma_start(out=dbg_gx[:], in_=gx[:]).then_inc(ld, 16)
                        sync.dma_start(out=dbg_ix[:],
                                       in_=idx_sb[:]).then_inc(ld, 16)
                        sync.dma_start(out=dbg_v16[:],
                                       in_=val16[:]).then_inc(ld, 16)
                        sync.wait_ge(ld, 16 * (n_tiles * repeat + 3))

            @block.vector
            def _(vector):
                for rep in range(repeat):
                    if margin:
                        if rep:
                            # val still read by rep-1 Act pipeline
                            vector.wait_ge(ac, 3 * rep)
                        vector.memset(gx[:], 0.0).then_inc(mz, 1)
                    for i in range(n_tiles):
                        g = rep * n_tiles + i
                        vector.wait_ge(ld, 16 * (g + 1))
                        vector.tensor_scalar_mul(bufs[g % NB][:],
                                                 bufs[g % NB][:],
                                                 S).then_inc(cp, 1)
                        if margin and i == 2:
                            # DVE's deep pipeline needs explicit same-engine
                            # waits between RAW-dependent back-to-back ops
                            # (dv chain), mirroring what Tile emits.
                            d0 = 4 * rep
                            vector.wait_ge(gt, 64 * (rep + 1))
                            nc.vector.tensor_scalar(
                                num[:], gx[:], -1.0, 1.0,
                                mybir.AluOpType.mult,
                                mybir.AluOpType.add).then_inc(dv, 1)
                            nc.vector.tensor_scalar_add(
                                den[:], gx[:], 1.0).then_inc(dv, 1)
                            vector.wait_ge(dv, d0 + 2)
                            nc.vector.reciprocal(den[:],
                                                 den[:]).then_inc(dv, 1)
                            vector.wait_ge(dv, d0 + 3)
                            nc.vector.tensor_tensor(
                                out=val[:], in0=num[:], in1=den[:],
                                op=mybir.AluOpType.mult).then_inc(dv, 1)

            if margin:

                @block.scalar
                def _(scalar):
                    for rep in range(repeat):
                        if rep:
                            # val16 still read by rep-1 scatters
                            scalar.wait_ge(sc, 64 * rep)
                        a0 = 3 * rep
                        scalar.wait_ge(dv, 4 * (rep + 1))
                        # same-engine ac chain: in-place Sqrt -> Arctan ->
                        # Copy are RAW-dependent on the Act pipeline too.
                        nc.scalar.activation(
                            val[:], val[:],
                            mybir.ActivationFunctionType.Sqrt).then_inc(ac, 1)
                        scalar.wait_ge(ac, a0 + 1)
                        nc.scalar.activation(
                            val[:], val[:],
                            mybir.ActivationFunctionType.Arctan).then_inc(ac, 1)
                        scalar.wait_ge(ac, a0 + 2)
                        nc.scalar.activation(
                            val16[:], val[:],
                            mybir.ActivationFunctionType.Copy,
                            bias=S * B, scale=-2.0 * S * A).then_inc(ac, 1)

                @block.gpsimd
                def _(gpsimd):
                    for rep in range(repeat):
                        gpsimd.wait_ge(ix, 16 * (rep + 1))
                        gpsimd.wait_ge(mz, rep + 1)
                        for r in range(ROW_CHUNKS):
                            nc.gpsimd.indirect_dma_start(
                                out=gx[:, r : r + 1],
                                out_offset=None,
                                in_=cosf[:],
                                in_offset=bass.IndirectOffsetOnAxis(
                                    ap=idx_sb[:, r : r + 1], axis=1
                                ),
                                bounds_check=NELEM - 1,
                                oob_is_err=False,
                            ).then_inc(gt, 16)
                        gpsimd.wait_ge(ac, 3 * (rep + 1))
                        gpsimd.wait_ge(st, 16 * n_tiles * (rep + 1))
                        for r in range(ROW_CHUNKS):
                            nc.gpsimd.indirect_dma_start(
                                out=out[:],
                                out_offset=bass.IndirectOffsetOnAxis(
                                    ap=idx_sb[:, r : r + 1], axis=1
                                ),
                                in_=val16[:, r : r + 1],
                                in_offset=None,
                                bounds_check=NELEM - 1,
                                oob_is_err=False,
                            ).then_inc(sc, 16)

    nc.compile()
    return nc


def _in_maps(cosine: np.ndarray, label: np.ndarray):
    cosine = np.asarray(cosine, dtype=np.float32)
    cosine16 = cosine.astype(BF16_NP)
    label = np.asarray(label)
    rows = np.arange(BATCH, dtype=np.int64)
    in_maps = []
    for c in range(NCORES):
        lo = c * SHARD
        shard_f = np.ascontiguousarray(cosine[:, lo : lo + SHARD])
        shard16 = np.ascontiguousarray(cosine16[:, lo : lo + SHARD])
        loc = label.astype(np.int64) - lo
        valid = (label != -1) & (loc >= 0) & (loc < SHARD)
        flat = np.where(valid, rows * SHARD + loc, OOB_SENTINEL).astype(np.int32)
        # device layout: idx[p, r] = flat[r*128 + p]
        idx_dev = np.ascontiguousarray(flat.reshape(ROW_CHUNKS, 128).T)
        in_maps.append({"cos16": shard16, "cosf": shard_f, "idx": idx_dev})
    return in_maps


def kernel(cosine: np.ndarray, label: np.ndarray) -> np.ndarray:
    global _NC, LAST_RESULT
    if _NC is None:
        _NC = _build_raw()
    res = run_bass_kernel_spmd(_NC, _in_maps(cosine, label),
                               core_ids=list(range(NCORES)))
    LAST_RESULT = res
    out16 = np.concatenate([res.results[c]["out"] for c in range(NCORES)], axis=1)
    return out16.astype(np.float32)
